# revision 47
# baseline (speedup 1.0000x reference)
"""BDH linear-attention TRN2 kernel v3 — fp8 DoubleRow matmuls, data-parallel
over batch on 8 cores.

Per-core program (core b handles batch b):
  A. LayerNorm -> xn (bf16); PE-transpose -> xnT fp8 [128,6,1024]; residue
     gate; per-d token sums for the read/write gates.
  B. k/v projections as fp8 DoubleRow matmuls vs host-packed weights; hub
     feature map relu(q)^1.5 in three passes:
       p1  ACT  Relu(psum/8)              -> t  (bf16)
       pAB DVE  bits(t) -> (3*(bits+A))>>2 = bits of t^0.75  (two 4x TSPs)
       pC  ACT/DVE  Square(SACT * t^0.75) -> fp8 (= CY * t^1.5)
     per-head state via DoubleRow over token-tile pairs; k-normalization
     via DR ones-column sums.  AllReduce in two halves (heads 0-3 + write
     gates, heads 4-7).  D-phase work for heads 0-3 is emitted mid-B.
  C. q projection (fp8 DR) + feature map -> qhatT fp8 [128,24,1024].
  D. m' = memT64 + (boa*wsum_h)*st (fp8); W'_h = a_h*(m'^T @ wo64) -> Wp fp8.
  E. psum = qhatT^T @ Wp (fp8 DR); out = residue/DESC*psum + (1-residue)*x.

Scales: w_in*8 (host, fp8); memT*64, w_out*64 (host); feature-map constant
CY and 64*64 folded into the residue multiplier.
"""
import numpy as np
import ml_dtypes

import concourse.mybir as mybir
import concourse.tile as tile
from concourse import bacc
from concourse.masks import make_identity
from concourse.bass_utils import run_bass_kernel_spmd

F32 = mybir.dt.float32
BF16 = mybir.dt.bfloat16
FP8 = mybir.dt.float8e4
AF = mybir.ActivationFunctionType
OP = mybir.AluOpType
DR = mybir.MatmulPerfMode.DoubleRow
U16 = mybir.dt.uint16

B, N, D, H = 8, 1024, 768, 8
S = 3072
HD = 384
NT = N // 128        # 8 token tiles
KC = D // 128        # 6 contraction chunks
SC = S // 128        # 24 sparse chunks
EPS = 1e-6
LN_EPS = 1e-5
PERSIST = 0.95
N_CORES = 8

SW = 8.0                     # host prescale of w_in
MSC = 64.0                   # host prescale of memT / w_out
# feature map (bit-hack sqrt): t = max(psum, thr)*2^42 bf16; s = bits(t)>>1
# = C*sqrt(t); y = t*s -> fp8 = SQ_C * t^1.5, SQ_C folded into the descale.
SQ_PRE = 2.0 ** 42
SQ_C = 0.7199236
KTHR = SW * EPS              # feature-map clamp threshold (scaled)
OUT_DESCALE = SQ_C * SW ** 1.5 * MSC * MSC
BOA = (1.0 - PERSIST) / (N_CORES * N_CORES * SW * PERSIST)

CC_HALF = 4 * 3 * 128 * HD   # bf16 elements per half (4 heads)
CC_LEN_A = CC_HALF + 8
CC_LEN_B = CC_HALF
HBLK = 3 * 128 * HD          # per-head cc elements

NP_FP8 = ml_dtypes.float8_e4m3
NP_BF16 = ml_dtypes.bfloat16


def build_program(ln_trivial, b_in_zero, b_out_zero, single_core=False):
    nc = bacc.Bacc("TRN2", target_bir_lowering=False, debug=False,
                   num_devices=1 if single_core else N_CORES)

    x_d = nc.dram_tensor("x", [N, D], F32, kind="ExternalInput")
    wkv_d = nc.dram_tensor("wkv", [128, 96 * 384], FP8, kind="ExternalInput")
    wq_d = nc.dram_tensor("wq", [128, 144 * 128], FP8, kind="ExternalInput")
    wo_d = nc.dram_tensor("wo", [128, 48 * 384], FP8, kind="ExternalInput")
    mem_d = nc.dram_tensor("memT64", [128, 24 * 384], BF16, kind="ExternalInput")
    wg16_d = nc.dram_tensor("wg16", [128, KC * 16], BF16, kind="ExternalInput")
    b_rg_d = nc.dram_tensor("b_rg", [H], F32, kind="ExternalInput")
    b_wg_d = nc.dram_tensor("b_wg", [H], F32, kind="ExternalInput")
    wres16_d = nc.dram_tensor("wres16", [D], BF16, kind="ExternalInput")
    b_res_d = nc.dram_tensor("b_res", [1], F32, kind="ExternalInput")
    if not ln_trivial:
        ln_g_d = nc.dram_tensor("ln_g", [D], F32, kind="ExternalInput")
        ln_b_d = nc.dram_tensor("ln_b", [D], F32, kind="ExternalInput")
    if not b_in_zero:
        bq8_d = nc.dram_tensor("bq8", [128, SC], F32, kind="ExternalInput")
        bkv8_d = nc.dram_tensor("bkv8", [16, HD], F32, kind="ExternalInput")
    if not b_out_zero:
        b_out_d = nc.dram_tensor("b_out", [D], F32, kind="ExternalInput")
    out_d = nc.dram_tensor("out", [N, D], F32, kind="ExternalOutput")

    with tile.TileContext(nc) as tc:
        with (
            tc.tile_pool(name="const", bufs=1) as const,
            tc.tile_pool(name="resid", bufs=1) as resid,
            tc.tile_pool(name="wtop", bufs=1) as wtop,
            tc.tile_pool(name="scrD", bufs=1) as scrD,
            tc.tile_pool(name="ccdram", bufs=1, space="DRAM") as ccdram,
        ):

            # ---------------- constants ------------------------------------
            ident = const.tile([128, 128], BF16)
            make_identity(nc, ident[:])
            ones_col = const.tile([128, 1], BF16)
            nc.vector.memset(ones_col[:], 1.0)
            ones8t = const.tile([128, NT, 32], FP8)
            nc.vector.memset(ones8t[:], 1.0)
            lneps_col = const.tile([128, 1], F32)
            nc.vector.memset(lneps_col[:], LN_EPS)
            # warm the sqrt ACT table (includes relu/square/copy) early
            warm = const.tile([128, 1], F32)
            nc.scalar.activation(warm[:], lneps_col[:], AF.Sqrt)
            wres_b = const.tile([128, D], BF16)
            nc.sync.dma_start(wres_b[:], wres16_d.ap().partition_broadcast(128))
            bres_b = const.tile([128, 1], F32)
            nc.sync.dma_start(bres_b[:], b_res_d.ap().partition_broadcast(128))
            wg_sb = const.tile([128, KC, 16], BF16)
            nc.sync.dma_start(wg_sb[:],
                              wg16_d.ap().rearrange("p (c g) -> p c g", c=KC))
            gbias = const.tile([1, 16], F32)
            nc.sync.dma_start(gbias[:, 0:8], b_rg_d.ap().partition_broadcast(1))
            nc.sync.dma_start(gbias[:, 8:16], b_wg_d.ap().partition_broadcast(1))
            if not ln_trivial:
                lng_b = const.tile([128, D], F32)
                nc.gpsimd.dma_start(lng_b[:], ln_g_d.ap().partition_broadcast(128))
                lnb_b = const.tile([128, D], F32)
                nc.gpsimd.dma_start(lnb_b[:], ln_b_d.ap().partition_broadcast(128))
            if not b_in_zero:
                bq_sb = const.tile([128, SC], F32)
                nc.sync.dma_start(bq_sb[:], bq8_d[:, :])
                bkv_sb = const.tile([128, 16, HD], F32)
                nc.sync.dma_start(bkv_sb[:], bkv8_d.ap().partition_broadcast(128))
            if not b_out_zero:
                bout_b = const.tile([128, D], F32)
                nc.sync.dma_start(bout_b[:], b_out_d.ap().partition_broadcast(128))

            residue = resid.tile([128, NT], F32)
            rdiv = resid.tile([128, NT], F32)
            onemr = resid.tile([128, NT], F32)
            rlogs = resid.tile([128, NT], F32)
            gates_sb = resid.tile([1, 16], F32)
            xsum_sb = resid.tile([128, KC], BF16)
            xnT = wtop.tile([128, KC, N], FP8)
            qhatT = wtop.tile([128, SC, N], FP8)
            WpT = wtop.tile([128, SC, D], FP8)
            wo_sb = wtop.tile([128, 48, 384], FP8)
            stA_t = scrD.tile([128, 12, 384], BF16)
            mA_t = scrD.tile([128, 12, 384], FP8)
            memA_t = scrD.tile([128, 12, 384], BF16)
            stB1_t = scrD.tile([128, 6, 384], BF16)
            mB1_t = scrD.tile([128, 6, 384], FP8)
            memB1_t = scrD.tile([128, 6, 384], BF16)

            cc_in_a = ccdram.tile([CC_LEN_A], BF16)
            cc_in_b1 = ccdram.tile([2 * HBLK], BF16)
            cc_in_b2 = ccdram.tile([2 * HBLK], BF16)
            cc_out_a = ccdram.tile([CC_LEN_A], BF16,
                                   addr_space="Local" if single_core else "Shared")
            cc_out_b1 = ccdram.tile([2 * HBLK], BF16,
                                    addr_space="Local" if single_core else "Shared")
            cc_out_b2 = ccdram.tile([2 * HBLK], BF16,
                                    addr_space="Local" if single_core else "Shared")
            ab_dram = ccdram.tile([16], F32)
            zr_dram = ccdram.tile([H * HD], BF16)

            def do_collective(cin, cout, clen):
                if single_core:
                    nfull = (clen // 128) * 128
                    nc.sync.dma_start(
                        cout[0:nfull].rearrange("(p f) -> p f", p=128),
                        cin[0:nfull].rearrange("(p f) -> p f", p=128))
                    if clen > nfull:
                        nc.sync.dma_start(cout[nfull:clen],
                                          cin[nfull:clen])
                else:
                    nc.gpsimd.collective_compute(
                        "AllReduce", OP.add,
                        replica_groups=[list(range(N_CORES))],
                        ins=[cin.opt()], outs=[cout.opt()])

            def d_prep():
                wsum16 = scrD.tile([1, 8], BF16)
                nc.sync.dma_start(wsum16[:], cc_out_a[CC_HALF:CC_HALF + 8])
                wsum = scrD.tile([1, 8], F32)
                nc.vector.tensor_copy(wsum[:], wsum16[:])
                ab = scrD.tile([1, 16], F32)
                nc.vector.tensor_scalar_mul(ab[:, 0:8], gates_sb[:, 0:8],
                                            PERSIST)
                nc.vector.tensor_scalar_mul(ab[:, 8:16], wsum[:], BOA)
                nc.sync.dma_start(ab_dram[:], ab[:].opt())
                absb = scrD.tile([128, 16], F32)
                nc.sync.dma_start(absb[:], ab_dram[:].partition_broadcast(128))
                return absb

            def d_read(hs, st_t):
                if hs == 0:
                    nc.sync.dma_start(
                        st_t[:],
                        cc_out_a[0:CC_HALF].rearrange("(a p m) -> p a m",
                                                      a=12, p=128))
                    return
                for i, cout in enumerate((cc_out_b1, cc_out_b2)):
                    nc.sync.dma_start(
                        st_t[:, 6 * i:6 * i + 6, :],
                        cout[0:2 * HBLK].rearrange("(a p m) -> p a m",
                                                   a=6, p=128))

            def d_piece(h, hh, absb, st_t, m_t, mem_t, ps_wp):
                m_eng = nc.vector
                with nc.allow_low_precision(reason="m' fp8"):
                    m_eng.scalar_tensor_tensor(
                        m_t[:, hh * 3:hh * 3 + 3, :],
                        st_t[:, hh * 3:hh * 3 + 3, :],
                        absb[:, 8 + h:9 + h],
                        mem_t[:, hh * 3:hh * 3 + 3, :],
                        OP.mult, OP.add)
                for dc in range(3):
                    for jb in range(2):
                        pwp = ps_wp.tile([128, HD], F32, tag="wp")
                        nc.tensor.matmul(
                            pwp[:],
                            m_t[:, hh * 3:hh * 3 + 2,
                                dc * 128:(dc + 1) * 128],
                            wo_sb[:, h * 6 + jb * 3:
                                  h * 6 + jb * 3 + 2, :],
                            start=True, stop=False, perf_mode=DR)
                        nc.tensor.matmul(
                            pwp[:],
                            m_t[:, hh * 3 + 2, dc * 128:(dc + 1) * 128],
                            wo_sb[:, h * 6 + jb * 3 + 2, :],
                            start=False, stop=True)
                        dst = WpT[:, h * 3 + dc, jb * 384:(jb + 1) * 384]
                        with nc.allow_low_precision(reason="Wp fp8"):
                            if (dc + jb) % 2 == 0:
                                nc.scalar.mul(dst, pwp[:], absb[:, h:h + 1])
                            else:
                                nc.vector.tensor_scalar_mul(
                                    dst, pwp[:], absb[:, h:h + 1])

            def fmap_shift(s16, t16):
                # s = bitcast(bits(t) >> 1) = C*sqrt(t): one 4x TSP
                nc.vector.tensor_scalar(
                    s16[:].bitcast(U16), t16[:].bitcast(U16),
                    1, None, OP.logical_shift_right)

            def fmap_mul(dst, t16, s16, eng):
                # dst fp8 = t * s = SQ_C * t^1.5
                with nc.allow_low_precision(reason="fmap fp8"):
                    e = nc.gpsimd if eng == "pool" else nc.vector
                    e.tensor_mul(dst, t16[:], s16[:])

            def q_block2(sc, scr, psq):
                t16 = scr.tile([128, 2, 512], BF16, tag="t16")
                for nh in range(2):
                    pq = psq.tile([128, 512], F32, tag="q")
                    for i in range(3):
                        nc.tensor.matmul(
                            pq[:],
                            wq_sb[:, sc * 6 + 2 * i:sc * 6 + 2 * i + 2, :],
                            xnT[:, 2 * i:2 * i + 2, nh * 512:(nh + 1) * 512],
                            start=(i == 0), stop=(i == 2), perf_mode=DR)
                    if b_in_zero:
                        nc.scalar.activation(t16[:, nh, :], pq[:], AF.Relu,
                                             scale=SQ_PRE)
                    else:
                        with nc.allow_low_precision(reason="fmap"):
                            nc.vector.tensor_scalar(t16[:, nh, :], pq[:],
                                                    bq_sb[:, sc:sc + 1],
                                                    KTHR, OP.add, OP.max)
                            nc.vector.tensor_scalar_mul(
                                t16[:, nh, :], t16[:, nh, :], SQ_PRE)
                s16 = scr.tile([128, 2, 512], BF16, tag="s16")
                fmap_shift(s16, t16)
                fmap_mul(qhatT[:, sc, :], t16, s16,
                         "dve" if sc % 3 == 2 else "pool")

            # ================= phases A & B ================================
            with (
                tc.tile_pool(name="wkvp", bufs=1) as wkvp,
                tc.tile_pool(name="kvp", bufs=3) as kvp,
                tc.tile_pool(name="scrB", bufs=2) as scrB,
                tc.tile_pool(name="ps_kv", bufs=3, space="PSUM") as ps_kv,
            ):
                wkv_sb = wkvp.tile([128, 96, 384], FP8)
                wkv_r = wkv_d.ap().rearrange("p (a m) -> p a m", a=96)
                for hw in range(H):
                    nc.sync.dma_start(
                        wkv_sb[:, hw * 12:(hw + 1) * 12, :],
                        wkv_r[:, hw * 12:(hw + 1) * 12, :])
                wq_sb = wkvp.tile([128, 144, 128], FP8)
                wq_r = wq_d.ap().rearrange("p (a m) -> p a m", a=144)
                mem_r = mem_d.ap().rearrange("p (a m) -> p a m", a=24)

                wo_r = wo_d.ap().rearrange("p (a m) -> p a m", a=48)

                def late_loads(h):
                    if h == 0:
                        nc.sync.dma_start(wq_sb[:, 0:72, :], wq_r[:, 0:72, :])
                        nc.sync.dma_start(wq_sb[:, 72:144, :],
                                          wq_r[:, 72:144, :])
                    elif h == 1:
                        nc.sync.dma_start(memA_t[:], mem_r[:, 0:12, :])
                    elif h == 2:
                        nc.sync.dma_start(wo_sb[:, 0:24, :],
                                          wo_r[:, 0:24, :])
                        nc.sync.dma_start(wo_sb[:, 24:48, :],
                                          wo_r[:, 24:48, :])
                    elif h == 3:
                        nc.sync.dma_start(memB1_t[:], mem_r[:, 12:18, :])

                kv_tiles = {}

                def kv_proj(h, t, half, dst_ps):
                    base = (h * 2 + half) * 6
                    for i in range(3):
                        nc.tensor.matmul(
                            dst_ps[:],
                            xnT[:, 2 * i:2 * i + 2, t * 128:(t + 1) * 128],
                            wkv_sb[:, base + 2 * i:base + 2 * i + 2, :],
                            start=(i == 0), stop=(i == 2), perf_mode=DR)

                def kv_pair(h, u):
                    # k/v for token tiles 2u, 2u+1; feature map via
                    # relu -> bits-pow-0.75 -> square, batched per pair
                    if u == 0:
                        kv_tiles[h] = (
                            kvp.tile([128, NT, HD], FP8, tag="khat",
                                     name=f"khat{h}"),
                            kvp.tile([128, NT, HD], FP8, tag="vaug",
                                     name=f"vaug{h}"),
                        )
                    khat_h, v_h = kv_tiles[h]
                    t16 = scrB.tile([128, 2, HD], BF16, tag="t16")
                    for j in range(2):
                        t = 2 * u + j
                        pkv = ps_kv.tile([128, HD], F32, tag="kv")
                        kv_proj(h, t, 0, pkv)
                        src = pkv
                        if not b_in_zero:
                            pb = scrB.tile([128, HD], F32, tag="scr")
                            nc.vector.tensor_add(
                                pb[:], pkv[:], bkv_sb[:, h * 2, :])
                            src = pb
                        if b_in_zero:
                            nc.scalar.activation(t16[:, j, :], src[:],
                                                 AF.Relu, scale=SQ_PRE)
                        else:
                            with nc.allow_low_precision(reason="fmap"):
                                nc.vector.tensor_scalar(t16[:, j, :], src[:],
                                                        KTHR, SQ_PRE,
                                                        OP.max, OP.mult)
                    s16 = scrB.tile([128, 2, HD], BF16, tag="s16")
                    fmap_shift(s16, t16)
                    fmap_mul(khat_h[:, 2 * u:2 * u + 2, :], t16, s16, "pool")
                    for j in range(2):
                        t = 2 * u + j
                        pkv = ps_kv.tile([128, HD], F32, tag="kv")
                        kv_proj(h, t, 1, pkv)
                        src = pkv
                        if not b_in_zero:
                            pb = scrB.tile([128, HD], F32, tag="scr")
                            nc.vector.tensor_add(
                                pb[:], pkv[:], bkv_sb[:, h * 2 + 1, :])
                            src = pb
                        with nc.allow_low_precision(reason="v fp8"):
                            if t % 2 == 1:
                                nc.vector.tensor_copy(v_h[:, t, :], src[:])
                            else:
                                nc.scalar.copy(v_h[:, t, :], src[:])

                # ---------------- phase A: LayerNorm + transpose ----------
                with (
                    tc.tile_pool(name="lnp", bufs=2) as lnp,
                    tc.tile_pool(name="xa", bufs=3) as xa,
                    tc.tile_pool(name="ps_tp", bufs=2, space="PSUM") as ps_tp,
                    tc.tile_pool(name="ps_xs", bufs=1, space="PSUM") as ps_xs,
                    tc.tile_pool(name="ps_g", bufs=1, space="PSUM") as ps_g,
                ):
                    xsum_ps = ps_xs.tile([128, KC], F32)
                    for t in range(NT):
                        x_t = xa.tile([128, D], F32, tag="x")
                        nc.sync.dma_start(x_t[:], x_d[t * 128:(t + 1) * 128, :])
                        stats = lnp.tile([128, 3, 6], F32, tag="stats")
                        for g in range(3):
                            nc.vector.bn_stats(
                                stats[:, g, :],
                                x_t[:, g * 256:(g + 1) * 256])
                        mv = lnp.tile([128, 2], F32, tag="mv")
                        nc.vector.bn_aggr(mv[:], stats[:])
                        sq = lnp.tile([128, 1], F32, tag="sq")
                        nc.scalar.activation(sq[:], mv[:, 1:2], AF.Sqrt,
                                             bias=lneps_col[:], scale=1.0)
                        rstd = lnp.tile([128, 1], F32, tag="rstd")
                        nc.vector.reciprocal(rstd[:], sq[:])
                        xn = lnp.tile([128, D], BF16, tag="xn")
                        negmr = lnp.tile([128, 1], F32, tag="negmr")
                        nc.vector.tensor_scalar(negmr[:], mv[:, 0:1],
                                                rstd[:], -1.0,
                                                OP.mult, OP.mult)
                        nc.scalar.activation(xn[:], x_t[:],
                                             AF.Identity,
                                             bias=negmr[:],
                                             scale=rstd[:])
                        if not ln_trivial:
                            nc.vector.tensor_mul(xn[:], xn[:], lng_b[:])
                            nc.vector.tensor_add(xn[:], xn[:], lnb_b[:])
                        # residue gate logit (DVE accumulate); sigmoids are
                        # batched at the end of A (ACT table locality)
                        scr = lnp.tile([128, D], BF16, tag="scr")
                        nc.vector.scalar_tensor_tensor(
                            scr[:], xn[:], 0.0, wres_b[:], OP.add, OP.mult,
                            accum_out=rlogs[:, t:t + 1])
                        # token-sums per d for the gates
                        for c in range(KC):
                            nc.tensor.matmul(
                                xsum_ps[:, c:c + 1],
                                xn[:, c * 128:(c + 1) * 128], ones_col[:],
                                start=(t == 0), stop=(t == NT - 1))
                        # transpose to xnT (fp8)
                        for g in range(2):
                            tp = ps_tp.tile([128, 3, 128], BF16, tag="tp")
                            for c3 in range(3):
                                nc.tensor.transpose(
                                    tp[:, c3, :],
                                    xn[:, (g * 3 + c3) * 128:
                                       (g * 3 + c3 + 1) * 128],
                                    ident[:])
                            dst = xnT[:, g * 3:g * 3 + 3,
                                      t * 128:(t + 1) * 128]
                            nc.scalar.copy(dst, tp[:])
                        # head-0 k/v fills the PE while LN streams
                        if t % 2 == 1:
                            kv_pair(0, t // 2)

                    # batched residue sigmoids, then the gate sigmoids
                    nc.scalar.activation(residue[:], rlogs[:], AF.Sigmoid,
                                         bias=bres_b[:], scale=1.0)
                    nc.vector.tensor_copy(xsum_sb[:], xsum_ps[:])
                    gps = ps_g.tile([1, 16], F32)
                    for c in range(KC):
                        nc.tensor.matmul(gps[:], xsum_sb[:, c:c + 1],
                                         wg_sb[:, c, :],
                                         start=(c == 0), stop=(c == KC - 1))
                    glog = lnp.tile([1, 16], F32, tag="glog")
                    nc.vector.scalar_tensor_tensor(glog[:], gps[:], 1.0 / N,
                                                   gbias[:], OP.mult, OP.add)
                    nc.scalar.activation(gates_sb[:], glog[:], AF.Sigmoid)
                    wr16 = lnp.tile([1, 8], BF16, tag="wr16")
                    nc.vector.tensor_copy(wr16[:], gates_sb[:, 8:16])
                    nc.sync.dma_start(cc_in_a[CC_HALF:CC_HALF + 8],
                                      wr16[:].opt())
                    # E-phase residue scalars (off critical path)
                    nc.vector.tensor_scalar_mul(rdiv[:], residue[:],
                                                1.0 / OUT_DESCALE)
                    nc.vector.tensor_scalar(onemr[:], residue[:], -1.0, 1.0,
                                            OP.mult, OP.add)

                # ------------ phase B: kv + states + q --------------------
                with (
                    tc.tile_pool(name="stgp", bufs=1) as stgp,
                    tc.tile_pool(name="rbpool", bufs=2) as rbpool,
                    tc.tile_pool(name="scrC", bufs=2) as scrC,
                    tc.tile_pool(name="ps_z", bufs=1, space="PSUM") as ps_z,
                    tc.tile_pool(name="ps_st", bufs=1, space="PSUM") as ps_st,
                    tc.tile_pool(name="ps_q", bufs=2, space="PSUM") as ps_q,
                ):
                    rb_tiles = {}

                    def z_group(h):
                        khat_h, _ = kv_tiles[h]
                        zps = ps_z.tile([32, HD], F32, tag="z")
                        for u in range(NT // 2):
                            nc.tensor.matmul(
                                zps[:], ones8t[:, 2 * u:2 * u + 2, :],
                                khat_h[:, 2 * u:2 * u + 2, :],
                                start=(u == 0), stop=(u == NT // 2 - 1),
                                perf_mode=DR)
                        zrec16 = scrB.tile([1, HD], BF16, tag="zrec")
                        with nc.allow_low_precision(reason="1/z row"):
                            nc.vector.reciprocal(zrec16[:], zps[0:1, :])
                        # broadcast 1/z across partitions via DRAM round-trip
                        # (latency hidden by the one-head pipeline lead)
                        nc.sync.dma_start(zr_dram[h * HD:(h + 1) * HD],
                                          zrec16[:].opt())
                        rb = rbpool.tile([128, HD], BF16, tag="rb",
                                         name=f"rb{h}")
                        nc.sync.dma_start(
                            rb[:],
                            zr_dram[h * HD:(h + 1) * HD]
                            .partition_broadcast(128))
                        rb_tiles[h] = rb

                    def rb_state(h):
                        khat_h, v_h = kv_tiles.pop(h)
                        rb = rb_tiles.pop(h)
                        stx = stgp.tile([128, 3, HD], BF16, tag="stg",
                                        name=f"stg{h}")
                        for ec in range(3):
                            pst = ps_st.tile([128, HD], F32, tag="st")
                            for u in range(NT // 2):
                                nc.tensor.matmul(
                                    pst[:],
                                    v_h[:, 2 * u:2 * u + 2,
                                        ec * 128:(ec + 1) * 128],
                                    khat_h[:, 2 * u:2 * u + 2, :],
                                    start=(u == 0), stop=(u == NT // 2 - 1),
                                    perf_mode=DR)
                            nc.vector.scalar_tensor_tensor(
                                stx[:, ec, :], pst[:], 0.0,
                                rb[:], OP.add, OP.mult)
                        cin, slot = ((cc_in_a, h) if h < 4 else
                                     (cc_in_b1, h - 4) if h < 6 else
                                     (cc_in_b2, h - 6))
                        nc.sync.dma_start(
                            cin[slot * HBLK:(slot + 1) * HBLK]
                            .rearrange("(a p m) -> p a m", a=3, p=128),
                            stx[:])
                        if h == 3:
                            do_collective(cc_in_a, cc_out_a, CC_LEN_A)
                        elif h == 5:
                            do_collective(cc_in_b1, cc_out_b1, 2 * HBLK)
                        elif h == 7:
                            do_collective(cc_in_b2, cc_out_b2, 2 * HBLK)

                    absb = None
                    # software pipeline: kv two heads ahead, z one ahead;
                    # four q blocks interleaved per head from h==2
                    for u in range(NT // 2):
                        kv_pair(1, u)
                    z_group(0)
                    for h in range(H):
                        if h + 2 < H:
                            for u in range(NT // 2):
                                kv_pair(h + 2, u)
                        if h + 1 < H:
                            z_group(h + 1)
                        rb_state(h)
                        if h >= 2:
                            for k in range(4):
                                q_block2((h - 2) * 4 + k, scrC, ps_q)
                        late_loads(h)
                        if h == 5:
                            # collective half A landed long ago: get the
                            # D-phase scalars and st read going (DMA/DVE)
                            absb = d_prep()
                            d_read(0, stA_t)

                    # ---- B tail: D-phase heads 0-3, then 4-5 (cc B1
                    # landed at h==5) ---------------------------------------
                    for i in range(4):
                        d_piece(i, i, absb, stA_t, mA_t, memA_t, ps_z)
                    nc.sync.dma_start(
                        stB1_t[:],
                        cc_out_b1[0:2 * HBLK].rearrange(
                            "(a p m) -> p a m", a=6, p=128))
                    for hh in range(2):
                        d_piece(4 + hh, hh, absb, stB1_t, mB1_t, memB1_t,
                                ps_z)

            # ================= phase D (heads 6-7) and E ===================
            with tc.tile_pool(name="cp", bufs=1) as cp:
                stB2_t = cp.tile([128, 6, 384], BF16)
                mB2_t = cp.tile([128, 6, 384], FP8)
                memB2_t = cp.tile([128, 6, 384], BF16)
                nc.sync.dma_start(memB2_t[:], mem_r[:, 18:24, :])
                nc.sync.dma_start(
                    stB2_t[:],
                    cc_out_b2[0:2 * HBLK].rearrange(
                        "(a p m) -> p a m", a=6, p=128))
                with (
                    tc.tile_pool(name="ps_w2", bufs=2, space="PSUM") as ps_w2,
                ):
                    for hh in range(2):
                        d_piece(6 + hh, hh, absb, stB2_t, mB2_t, memB2_t,
                                ps_w2)

                # ============= phase E: output =============================
                with (
                    tc.tile_pool(name="ep", bufs=3) as ep,
                    tc.tile_pool(name="xe", bufs=3) as xe,
                    tc.tile_pool(name="ps_o", bufs=4, space="PSUM") as ps_o,
                ):
                    for t in range(NT):
                        x_t = xe.tile([128, D], F32, tag="x")
                        nc.sync.dma_start(x_t[:], x_d[t * 128:(t + 1) * 128, :])
                        pos = [ps_o.tile([128, HD], F32, tag="o",
                                         name=f"o{t}_{jb}")
                               for jb in range(2)]
                        for e in range(SC // 2):
                            for jb in range(2):
                                nc.tensor.matmul(
                                    pos[jb][:],
                                    qhatT[:, 2 * e:2 * e + 2,
                                          t * 128:(t + 1) * 128],
                                    WpT[:, 2 * e:2 * e + 2,
                                        jb * 384:(jb + 1) * 384],
                                    start=(e == 0), stop=(e == SC // 2 - 1),
                                    perf_mode=DR)
                        u = ep.tile([128, D], F32, tag="u")
                        nc.scalar.mul(u[:], x_t[:], onemr[:, t:t + 1])
                        if not b_out_zero:
                            u2 = ep.tile([128, D], F32, tag="u2")
                            nc.vector.scalar_tensor_tensor(
                                u2[:], bout_b[:], residue[:, t:t + 1], u[:],
                                OP.mult, OP.add)
                            u = u2
                        osb = ep.tile([128, D], F32, tag="osb")
                        for jb in range(2):
                            eng = nc.vector
                            eng.scalar_tensor_tensor(
                                osb[:, jb * 384:(jb + 1) * 384], pos[jb][:],
                                rdiv[:, t:t + 1],
                                u[:, jb * 384:(jb + 1) * 384],
                                OP.mult, OP.add)
                        nc.sync.dma_start(out_d[t * 128:(t + 1) * 128, :],
                                          osb[:])

    nc.compile()
    return nc


_PROGRAM_CACHE = {}


def _get_program(key):
    if key not in _PROGRAM_CACHE:
        _PROGRAM_CACHE[key] = build_program(*key)
    return _PROGRAM_CACHE[key]


def _pack_weights(w_in, w_out, memory):
    w8 = np.asarray(SW * w_in, NP_FP8)              # [768, 9216]
    # wkv: [p, h, half, i, j, c] -> [128, 96*384]
    wk = np.ascontiguousarray(
        w8[:, S:2 * S].reshape(3, 2, 128, H, HD).transpose(2, 3, 0, 1, 4))
    wv = np.ascontiguousarray(
        w8[:, 2 * S:3 * S].reshape(3, 2, 128, H, HD).transpose(2, 3, 0, 1, 4))
    wkv = np.stack([wk, wv], axis=2)                # [p, h, half, i, j, c]
    wkv = np.ascontiguousarray(wkv).reshape(128, 96 * 384)
    # wq: [p, sc, i, j, m] -> [128, 144*128]
    wq = np.ascontiguousarray(
        w8[:, 0:S].reshape(3, 2, 128, SC, 128).transpose(2, 3, 0, 1, 4))
    wq = wq.reshape(128, 144 * 128)
    # wo: [p, h, jb, ec, d] -> [128, 48*384]
    wo64 = np.asarray(MSC * w_out, NP_FP8)          # [3072, 768]
    wo = np.ascontiguousarray(
        wo64.reshape(H, 3, 128, 2, HD).transpose(2, 0, 3, 1, 4))
    wo = wo.reshape(128, 48 * 384)
    # memT64: [p, h, ec, d] -> [128, 24*384] bf16
    memT = np.ascontiguousarray(MSC * memory.transpose(0, 2, 1))  # [h, e, d]
    memb = np.asarray(memT, NP_BF16).reshape(H, 3, 128, HD)
    memb = np.ascontiguousarray(memb.transpose(2, 0, 1, 3)).reshape(128, 24 * 384)
    return wkv, wq, wo, memb


def kernel(x, memory, ln_g, ln_b, w_in, b_in, w_out, b_out,
           w_rg, b_rg, w_wg, b_wg, w_res, b_res):
    x = np.ascontiguousarray(np.asarray(x, dtype=np.float32))
    memory = np.asarray(memory, dtype=np.float32)
    ln_g = np.asarray(ln_g, dtype=np.float32)
    ln_b = np.asarray(ln_b, dtype=np.float32)
    w_in = np.ascontiguousarray(np.asarray(w_in, dtype=np.float32))
    b_in = np.asarray(b_in, dtype=np.float32)
    w_out = np.asarray(w_out, dtype=np.float32)
    b_out = np.asarray(b_out, dtype=np.float32)
    w_rg = np.asarray(w_rg, dtype=np.float32)
    b_rg = np.asarray(b_rg, dtype=np.float32)
    w_wg = np.asarray(w_wg, dtype=np.float32)
    b_wg = np.asarray(b_wg, dtype=np.float32)
    w_res = np.asarray(w_res, dtype=np.float32)
    b_res = np.asarray(b_res, dtype=np.float32)

    ln_trivial = bool(np.all(ln_g == 1.0) and np.all(ln_b == 0.0))
    b_in_zero = bool(np.all(b_in == 0.0))
    b_out_zero = bool(np.all(b_out == 0.0))

    nc = _get_program((ln_trivial, b_in_zero, b_out_zero))
    wkv, wq, wo, memb = _pack_weights(w_in, w_out, memory)

    wg16 = np.zeros((128, KC, 16), NP_BF16)
    wg16[:, :, 0:8] = w_rg.reshape(KC, 128, H).transpose(1, 0, 2)
    wg16[:, :, 8:16] = w_wg.reshape(KC, 128, H).transpose(1, 0, 2)
    shared = {
        "wkv": wkv, "wq": wq, "wo": wo, "memT64": memb,
        "wg16": np.ascontiguousarray(wg16).reshape(128, KC * 16),
        "b_rg": b_rg, "b_wg": b_wg,
        "wres16": np.asarray(w_res[:, 0], NP_BF16), "b_res": b_res,
    }
    if not ln_trivial:
        shared["ln_g"] = ln_g
        shared["ln_b"] = ln_b
    if not b_in_zero:
        bq8 = np.ascontiguousarray(
            (SW * b_in[0:S]).reshape(SC, 128).T.astype(np.float32))
        bkv8 = np.ascontiguousarray(
            (SW * b_in[S:3 * S]).reshape(2, H, HD).transpose(1, 0, 2)
            .reshape(16, HD).astype(np.float32))
        shared["bq8"] = bq8
        shared["bkv8"] = bkv8
    if not b_out_zero:
        shared["b_out"] = b_out

    in_maps = [{"x": x[b], **shared} for b in range(N_CORES)]
    res = run_bass_kernel_spmd(nc, in_maps, list(range(N_CORES)))
    return np.stack([res.results[b]["out"] for b in range(N_CORES)], axis=0)

